# revision 1
# baseline (speedup 1.0000x reference)
"""Per-patch softmax ("kernel activation") on Trainium2 via Bass/Tile.

Reference op: x:(16,64,256,256) f32, k=4. Unfold each (H,W) plane into
non-overlapping 4x4 patches, softmax over the 16 patch elements, fold back.

Strategy (data parallel over batch, 2 batches per core on 8 cores):
  - fp16 I/O: x is downcast to fp16 on the host, y is produced in fp16 and
    upcast on the host. Halves HBM traffic (the roofline) vs f32; softmax
    outputs in [0,1] keep absmax rel err ~1.5e-3, well under the 2e-2 gate.
  - SBUF tile = [128 partitions, NJ rows x 256 cols]; partition p holds NJ
    CONSECUTIVE image rows so every 4x4 patch lives inside one partition and
    each partition's DMA span is one contiguous NJ*512B chunk.
  - per tile: exp on ACT; patch sums via a 4-op pairwise tensor_add tree on
    DVE (TT runs 2x_1p on fp16, vs tensor_reduce which is 1x-only);
    reciprocal via the fast custom-DVE approx (f32, ~51 ULP); the per-patch
    reciprocal is broadcast-expanded over the patch-col axis by ACT (so the
    final multiply's innermost AP stride is 1 -> keeps DVE in 2x mode); the
    multiply writes back into the x tile, which is stored out on a queue
    that never blocks the ACT or SP instruction streams.
  - emission is software-pipelined with a tile skew: the ACT stream is
    [... exp(t), expandR(t-skew) ...] so expandR's wait on DVE's
    recip(t-skew) is already satisfied, and the DVE stream is [adds(t),
    recip(t), mul(t-skew)] likewise.
"""

import numpy as np

import concourse.bacc as bacc
import concourse.bass as bass
import concourse.tile as tile
from concourse import mybir
from concourse.bass_utils import run_bass_kernel_spmd

B, C, H, W = 16, 64, 256, 256
KP = 4                       # patch edge (the "k" input; hardcoded)
NCORES = 8
B_LOC = B // NCORES          # batches per core
ROWS = B_LOC * C * H         # 32768 DRAM rows per core
P = 128                      # SBUF partitions
NJ = 16                      # image rows per partition per tile (default)

F16 = mybir.dt.float16
F32 = mybir.dt.float32

_cached = {}


def _ap(t, off, dims):
    """View of tile t at element offset off with free dims [[stride, size], ...]."""
    return bass.AP(tensor=t.tensor, offset=t.offset + off, ap=[t.ap[0], *dims])


def _build(
    reps: int = 1,
    nj: int = NJ,
    store_q: str = "gpsimd",
    skew: int = 2,
    balance: str = "dve",
) -> bass.Bass:
    """Build the kernel.

    reps>1 re-runs the whole pass that many times over the same x/y (x is
    only read; y stores are FIFO per queue so the last pass wins) — used by
    test.py to measure steady-state per-pass device time as
    (t_reps - t_1) / (reps - 1) with dispatch overhead cancelled.

    store_q: which queue issues output DMAs ("gpsimd" = SWDGE on Pool,
    "scalar" = ACT HWDGE, "sync" = SP HWDGE).
    skew: tiles of delay between the front half (load/exp/sums/recip) and
    the back half (expand/mul/store) of the per-tile pipeline.
    """
    FREE = nj * W              # free elems per partition per tile
    Q = nj // KP               # patch-rows per partition per tile
    G = W // KP                # patch columns per image row (64)
    T = ROWS // (P * nj)       # tiles per pass

    nc = bacc.Bacc(trn_type="TRN2")
    x = nc.dram_tensor("x", [ROWS, W], F16, kind="ExternalInput")
    y = nc.dram_tensor("y", [ROWS, W], F16, kind="ExternalOutput")

    xv = x[:].rearrange("(t p j) w -> t p (j w)", p=P, j=nj)
    yv = y[:].rearrange("(t p j) w -> t p (j w)", p=P, j=nj)

    store_eng = {"gpsimd": nc.gpsimd, "scalar": nc.scalar, "sync": nc.sync}[store_q]
    # balance="pool": offload the two small tail adds and the broadcast
    # expand to the (otherwise idle) gpsimd engine, so DVE carries only
    # add1+add2+recip+mul and ACT carries only exp.
    tail_eng = nc.gpsimd if balance == "pool" else nc.vector

    with tile.TileContext(nc) as tc:
        with (
            tc.tile_pool(name="xp", bufs=3 + skew) as xp,
            tc.tile_pool(name="ep", bufs=2 + skew) as ep,
            tc.tile_pool(name="h1p", bufs=2) as h1p,
            tc.tile_pool(name="h2p", bufs=2) as h2p,
            tc.tile_pool(name="h3p", bufs=2) as h3p,
            tc.tile_pool(name="sp", bufs=2) as sp,
            tc.tile_pool(name="rp", bufs=1 + skew) as rp,
            tc.tile_pool(name="Rp", bufs=2) as Rp,
        ):

            def front(t):
                """load -> exp -> patch-sum tree -> reciprocal (per tile t)."""
                xt = xp.tile([P, FREE], F16)
                nc.sync.dma_start(out=xt, in_=xv[t])

                et = ep.tile([P, FREE], F16)
                nc.scalar.activation(
                    out=et, in_=xt, func=mybir.ActivationFunctionType.Exp
                )

                # flat layout inside a partition: [q(Q), a(KP), g(G), b(KP)]
                # add1: pair rows a: (0+1),(2+3) -> h1 [q, c(2), g, b]
                h1 = h1p.tile([P, FREE // 2], F16)
                nc.vector.tensor_add(
                    _ap(h1, 0, [[FREE // 2 // Q, Q], [W, 2], [KP, G], [1, KP]]),
                    _ap(et, 0, [[FREE // Q, Q], [2 * W, 2], [KP, G], [1, KP]]),
                    _ap(et, W, [[FREE // Q, Q], [2 * W, 2], [KP, G], [1, KP]]),
                )
                # add2: fold the two row-pairs -> h2 [q, g, b]
                h2 = h2p.tile([P, FREE // 4], F16)
                nc.vector.tensor_add(
                    _ap(h2, 0, [[W, Q], [KP, G], [1, KP]]),
                    _ap(h1, 0, [[2 * W, Q], [KP, G], [1, KP]]),
                    _ap(h1, W, [[2 * W, Q], [KP, G], [1, KP]]),
                )
                # add3: pair cols b: (0+2),(1+3) -> h3 [q, g, e(2)]
                h3 = h3p.tile([P, FREE // 8], F16)
                tail_eng.tensor_add(
                    _ap(h3, 0, [[2 * G, Q], [2, G], [1, 2]]),
                    _ap(h2, 0, [[W, Q], [KP, G], [1, 2]]),
                    _ap(h2, 2, [[W, Q], [KP, G], [1, 2]]),
                )
                # add4: final fold -> S [q, g] in f32 (recip needs f32)
                s32 = sp.tile([P, Q * G], F32)
                tail_eng.tensor_add(
                    _ap(s32, 0, [[G, Q], [1, G]]),
                    _ap(h3, 0, [[2 * G, Q], [2, G]]),
                    _ap(h3, 1, [[2 * G, Q], [2, G]]),
                )
                r32 = rp.tile([P, Q * G], F32)
                nc.vector.reciprocal_approx_fast(out=r32, in_=s32)
                return xt, et, r32, t

            def back(st):
                """expand recip over b (ACT) -> multiply (DVE) -> store."""
                xt, et, r32, t = st
                R = Rp.tile([P, Q * G * KP], F16)
                if balance == "cce":
                    # No DVE multiply at all: store R (broadcast over the
                    # patch-row axis) into y, then store exp with a CCE
                    # elementwise-multiply accumulate: y = exp * R.
                    nc.scalar.copy(
                        out=_ap(R, 0, [[G * KP, Q], [KP, G], [1, KP]]),
                        in_=_ap(r32, 0, [[G, Q], [1, G], [0, KP]]),
                    )
                    y3 = yv[t].rearrange("p (q a w) -> p q a w", q=Q, a=KP)
                    for a in range(KP):
                        nc.gpsimd.dma_start(
                            out=y3[:, :, a],
                            in_=_ap(R, 0, [[G * KP, Q], [1, G * KP]]),
                        )
                    nc.gpsimd.dma_start(
                        out=yv[t], in_=et, accum_op=mybir.AluOpType.mult
                    )
                    return
                if balance == "pool":
                    nc.gpsimd.tensor_copy(
                        _ap(R, 0, [[G * KP, Q], [KP, G], [1, KP]]),
                        _ap(r32, 0, [[G, Q], [1, G], [0, KP]]),
                    )
                else:
                    nc.scalar.copy(
                        out=_ap(R, 0, [[G * KP, Q], [KP, G], [1, KP]]),
                        in_=_ap(r32, 0, [[G, Q], [1, G], [0, KP]]),
                    )
                nc.vector.tensor_mul(
                    _ap(xt, 0, [[FREE // Q, Q], [W, KP], [KP, G], [1, KP]]),
                    _ap(et, 0, [[FREE // Q, Q], [W, KP], [KP, G], [1, KP]]),
                    _ap(R, 0, [[G * KP, Q], [0, KP], [KP, G], [1, KP]]),
                )
                store_eng.dma_start(out=yv[t], in_=xt)

            pending = []
            for _ in range(reps):
                for t in range(T):
                    pending.append(front(t))
                    if len(pending) > skew:
                        back(pending.pop(0))
            for st in pending:
                back(st)

    nc.compile()
    return nc


def _get_nc(reps: int = 1, **kw) -> bass.Bass:
    key = (reps, tuple(sorted(kw.items())))
    if key not in _cached:
        _cached[key] = _build(reps, **kw)
    return _cached[key]


def _run(x_np: np.ndarray, **kwargs):
    nc = _get_nc()
    x16 = np.ascontiguousarray(x_np.reshape(NCORES, ROWS, W)).astype(np.float16)
    in_maps = [{"x": x16[i]} for i in range(NCORES)]
    res = run_bass_kernel_spmd(nc, in_maps, core_ids=list(range(NCORES)), **kwargs)
    out = np.concatenate(
        [
            np.asarray(r["y"]).astype(np.float32).reshape(B_LOC, C, H, W)
            for r in res.results
        ],
        axis=0,
    )
    return out, res


def kernel(x, k) -> np.ndarray:
    assert int(k) == KP, f"kernel hardcodes k={KP}, got {k}"
    x_np = np.asarray(x, dtype=np.float32)
    assert x_np.shape == (B, C, H, W)
    out, _ = _run(x_np)
    return out



# revision 2
# speedup vs baseline: 2.9492x; 2.9492x over previous
"""Per-patch softmax ("kernel activation") on Trainium2 via Bass/Tile.

Reference op: x:(16,64,256,256) f32, k=4. Unfold each (H,W) plane into
non-overlapping 4x4 patches, softmax over the 16 patch elements, fold back.

Strategy (data parallel over batch, 2 batches per core on 8 cores):
  - fp16 I/O: x is downcast to fp16 on the host, y is produced in fp16 and
    upcast on the host. Halves HBM traffic (the roofline) vs f32; softmax
    outputs in [0,1] keep absmax rel err ~1.1e-3, well under the 2e-2 gate.
  - SBUF tile = [128 partitions, 32 rows x 256 cols]; partition p holds 32
    CONSECUTIVE image rows so every 4x4 patch lives inside one partition and
    each partition's DMA span is one contiguous 16KB chunk (8 tiles/pass:
    fewer, larger DMAs and fewer instructions than the 16-row tiling).
  - Engine assignment (HW-measured rates, not cost-model):
      ACT:  exp (1 elem/lane/cyc, dtype-independent) + the reciprocal
            broadcast-expand copy (ACT is the only engine that does the
            f32->f16 strided expand at full rate).
      DVE:  the whole add tree + reciprocal + final multiply, all in fp16
            2x mode. The multiply keeps a 0-stride AP on the patch-row axis
            of R -- measured to still run at 2x (only the innermost stride
            matters), so no 4-way split and no expanded-R materialization.
      Pool: NOTHING. The DVE 2x perf mode and GPSIMD arbitrate an exclusive
            SBUF port lock per instruction, so any Pool tensor work (or
            SWDGE store descriptor generation) serializes against DVE 2x
            ops. Offloading adds to Pool measured 40% SLOWER end-to-end.
  - Both loads and stores issue from the SP (sync) HWDGE queue; stores are
    emitted with a 4-tile skew after the front half so their semaphore waits
    are pre-satisfied and never stall the load stream.
  - Deep buffering (xp=5, ep=4 bufs) + skew=4 measured best on HW: the
    kernel is DMA-bound (~33.5 MB/core/pass), so DMA must never idle.
"""

import numpy as np

import concourse.bacc as bacc
import concourse.bass as bass
import concourse.tile as tile
from concourse import mybir
from concourse.bass_utils import run_bass_kernel_spmd

B, C, H, W = 16, 64, 256, 256
KP = 4                       # patch edge (the "k" input; hardcoded)
NCORES = 8
B_LOC = B // NCORES          # batches per core
ROWS = B_LOC * C * H         # 32768 DRAM rows per core
P = 128                      # SBUF partitions
NJ = 32                      # image rows per partition per tile

F16 = mybir.dt.float16
F32 = mybir.dt.float32

_cached = {}


def _ap(t, off, dims):
    """View of tile t at element offset off with free dims [[stride, size], ...]."""
    return bass.AP(tensor=t.tensor, offset=t.offset + off, ap=[t.ap[0], *dims])


def _build(reps: int = 1, nj: int = NJ, skew: int = 4) -> bass.Bass:
    """Build the kernel.

    reps>1 re-runs the whole pass that many times over the same x/y (x is
    only read; y stores are FIFO per queue so the last pass wins) -- used by
    test.py to measure steady-state per-pass device time as
    (t_reps - t_1) / (reps - 1) with dispatch overhead cancelled.
    """
    FREE = nj * W              # free elems per partition per tile
    Q = nj // KP               # patch-rows per partition per tile
    G = W // KP                # patch columns per image row (64)
    T = ROWS // (P * nj)       # tiles per pass

    nc = bacc.Bacc(trn_type="TRN2")
    x = nc.dram_tensor("x", [ROWS, W], F16, kind="ExternalInput")
    y = nc.dram_tensor("y", [ROWS, W], F16, kind="ExternalOutput")

    xv = x[:].rearrange("(t p j) w -> t p (j w)", p=P, j=nj)
    yv = y[:].rearrange("(t p j) w -> t p (j w)", p=P, j=nj)

    with tile.TileContext(nc) as tc:
        with (
            tc.tile_pool(name="xp", bufs=5) as xp,
            tc.tile_pool(name="ep", bufs=4) as ep,
            tc.tile_pool(name="h1p", bufs=2) as h1p,
            tc.tile_pool(name="h2p", bufs=2) as h2p,
            tc.tile_pool(name="h3p", bufs=2) as h3p,
            tc.tile_pool(name="sp", bufs=2) as sp,
            tc.tile_pool(name="rp", bufs=2 + skew) as rp,
            tc.tile_pool(name="Rp", bufs=2) as Rp,
        ):

            def front(t):
                """load -> exp -> patch-sum tree -> reciprocal (per tile t)."""
                xt = xp.tile([P, FREE], F16)
                nc.sync.dma_start(out=xt, in_=xv[t])

                et = ep.tile([P, FREE], F16)
                nc.scalar.activation(
                    out=et, in_=xt, func=mybir.ActivationFunctionType.Exp
                )

                # flat layout inside a partition: [q(Q), a(KP), g(G), b(KP)]
                # add1: pair rows a: (0+1),(2+3) -> h1 [q, c(2), g, b]
                h1 = h1p.tile([P, FREE // 2], F16)
                nc.vector.tensor_add(
                    _ap(h1, 0, [[FREE // 2 // Q, Q], [W, 2], [KP, G], [1, KP]]),
                    _ap(et, 0, [[FREE // Q, Q], [2 * W, 2], [KP, G], [1, KP]]),
                    _ap(et, W, [[FREE // Q, Q], [2 * W, 2], [KP, G], [1, KP]]),
                )
                # add2: fold the two row-pairs -> h2 [q, g, b]
                h2 = h2p.tile([P, FREE // 4], F16)
                nc.vector.tensor_add(
                    _ap(h2, 0, [[W, Q], [KP, G], [1, KP]]),
                    _ap(h1, 0, [[2 * W, Q], [KP, G], [1, KP]]),
                    _ap(h1, W, [[2 * W, Q], [KP, G], [1, KP]]),
                )
                # add3: pair cols b: (0+2),(1+3) -> h3 [q, g, e(2)]
                h3 = h3p.tile([P, FREE // 8], F16)
                nc.vector.tensor_add(
                    _ap(h3, 0, [[2 * G, Q], [2, G], [1, 2]]),
                    _ap(h2, 0, [[W, Q], [KP, G], [1, 2]]),
                    _ap(h2, 2, [[W, Q], [KP, G], [1, 2]]),
                )
                # add4: final fold -> S [q, g] in f32 (recip needs f32)
                s32 = sp.tile([P, Q * G], F32)
                nc.vector.tensor_add(
                    _ap(s32, 0, [[G, Q], [1, G]]),
                    _ap(h3, 0, [[2 * G, Q], [2, G]]),
                    _ap(h3, 1, [[2 * G, Q], [2, G]]),
                )
                r32 = rp.tile([P, Q * G], F32)
                nc.vector.reciprocal_approx_fast(out=r32, in_=s32)
                return xt, et, r32, t

            def back(st):
                """expand recip over b (ACT) -> multiply (DVE 2x) -> store."""
                xt, et, r32, t = st
                R = Rp.tile([P, Q * G * KP], F16)
                nc.scalar.copy(
                    out=_ap(R, 0, [[G * KP, Q], [KP, G], [1, KP]]),
                    in_=_ap(r32, 0, [[G, Q], [1, G], [0, KP]]),
                )
                nc.vector.tensor_mul(
                    _ap(xt, 0, [[FREE // Q, Q], [W, KP], [KP, G], [1, KP]]),
                    _ap(et, 0, [[FREE // Q, Q], [W, KP], [KP, G], [1, KP]]),
                    _ap(R, 0, [[G * KP, Q], [0, KP], [KP, G], [1, KP]]),
                )
                nc.sync.dma_start(out=yv[t], in_=xt)

            pending = []
            for _ in range(reps):
                for t in range(T):
                    pending.append(front(t))
                    if len(pending) > skew:
                        back(pending.pop(0))
            for st in pending:
                back(st)

    nc.compile()
    return nc


def _get_nc(reps: int = 1, **kw) -> bass.Bass:
    key = (reps, tuple(sorted(kw.items())))
    if key not in _cached:
        _cached[key] = _build(reps, **kw)
    return _cached[key]


def _run(x_np: np.ndarray, **kwargs):
    nc = _get_nc()
    x16 = np.ascontiguousarray(x_np.reshape(NCORES, ROWS, W)).astype(np.float16)
    in_maps = [{"x": x16[i]} for i in range(NCORES)]
    res = run_bass_kernel_spmd(nc, in_maps, core_ids=list(range(NCORES)), **kwargs)
    out = np.concatenate(
        [
            np.asarray(r["y"]).astype(np.float32).reshape(B_LOC, C, H, W)
            for r in res.results
        ],
        axis=0,
    )
    return out, res


def kernel(x, k) -> np.ndarray:
    assert int(k) == KP, f"kernel hardcodes k={KP}, got {k}"
    x_np = np.asarray(x, dtype=np.float32)
    assert x_np.shape == (B, C, H, W)
    out, _ = _run(x_np)
    return out
